# revision 24
# baseline (speedup 1.0000x reference)
# Trainium2 Bass kernel for nn_DifferentiableFeatureLayer.
#
# Math (per reference):
#   bw[b]   = full_series[starts[b]-W : starts[b]+T]            (B, W+T, C)
#   f_mean  = conv(bw, w1)/s1 ; m2 = conv(bw, w2)/s2
#   var2    = conv(bw^2, w2)/s2 - m2^2 ; f_std = sqrt(var2 + 1e-8)
#   out     = concat([x, BN(f_mean), BN(f_std)], -1)            (B, T, 3C)
# where conv is a per-channel sliding window of length W over time and BN
# normalizes per channel over (B, T).
#
# Sharding: by channel - core k owns channels [4k, 4k+4); BN is per channel so
# cores are independent (no collectives). Host extracts the runtime-indexed
# windows and passes x through.
#
# Device compute: sliding window = banded (Toeplitz) matmul in bf16 (PSUM
# accumulates fp32):
#   acc[b, 128q+r] = sum_p sum_kp T_p[kp, r] * G[kp, b, q+p]
# The std-feature Toeplitz has 1/s2 folded in, so acc2 = m2 directly and
# acc3 = E[w2 x^2]/s2; v = acc3 - m2^2; f_std = sqrt(v + 1e-8).
# The mean feature stays in "h-units" (h = s1*f_mean): BN(h/s1) is the affine
# a*h + b with a = gamma/sqrt(var_h + s1^2*eps), b = beta - mu_h*a, so 1/s1
# only ever enters through the constant C = s1^2*eps.
#
# BN stats: per-partition partial sums (DVE reduces + fused tensor_tensor_
# reduce accumulators) -> gpsimd partition_all_reduce -> replicated [128,16]
# sums -> short per-seg affine chain -> per-seg scalars applied straight out
# of PSUM/SBUF into a bf16 output tile (DVE/ACT/Pool split).
#
# Input DMA is 3 bf16 chunks (std toeplitz+G first, then mean toeplitz,
# consts last) so std convs start while mean data is still in flight.

import numpy as np
import ml_dtypes

import concourse.bass as bass
import concourse.bacc as bacc
import concourse.tile as tile
from concourse import mybir
from concourse import bass_isa
from concourse.bass_utils import run_bass_kernel_spmd

B, T, C = 16, 512, 32
W = 128
SERIES_LEN = 100000
WIN_MIN, WIN_MAX = 2.0, 64.0
SHARP = 1.0
BN_EPS = 1e-5
STD_EPS = 1e-8

NCORES = 8
CPC = C // NCORES          # channels per core = 4
Q = T // 128               # 4 time blocks
NB = B * Q                 # 64 matmul columns
NBT = B * T                # BN population per channel
F32 = mybir.dt.float32
BF16 = mybir.dt.bfloat16
MUL = mybir.AluOpType.mult
ADD = mybir.AluOpType.add
SUB = mybir.AluOpType.subtract
SQRT = mybir.ActivationFunctionType.Sqrt
SQUARE = mybir.ActivationFunctionType.Square
IDENT = mybir.ActivationFunctionType.Identity

BNP = ml_dtypes.bfloat16

# tg blob layout (bf16, [128, 2368]):
#   chunk A1 (cols 0:672):    T1k(c0) 256 | T1k(c1) 256 | G(c0) 80 | G(c1) 80
#   chunk A2 (cols 672:1344): same for c2, c3
#   chunk B  (cols 1344:2368): T0(c0..c3), 256 each
CHUNK = 672
TGW = 2 * CHUNK + 4 * 256  # 2368


def _sigmoid(x):
    out = np.empty_like(x)
    pos = x >= 0
    out[pos] = 1.0 / (1.0 + np.exp(-x[pos]))
    ex = np.exp(x[~pos])
    out[~pos] = ex / (1.0 + ex)
    return out


def _soft_window_weights(raw):
    # (C,) -> (W, C), float64 for host-side accuracy
    win = WIN_MIN + _sigmoid(raw.astype(np.float64)) * (WIN_MAX - WIN_MIN)
    age = np.arange(W, dtype=np.float64)[::-1]
    return _sigmoid(SHARP * (win[None, :] - age[:, None]))


def _toeplitz_pair(wt):
    # wt: (W,) -> (2, 128, 128) band matrices T_p[kp, r] = wt[128p + kp - r]
    kp = np.arange(128)[:, None]
    r = np.arange(128)[None, :]
    out = np.zeros((2, 128, 128), np.float64)
    for p in range(2):
        idx = 128 * p + kp - r
        valid = (idx >= 0) & (idx < W)
        out[p] = np.where(valid, wt[np.clip(idx, 0, W - 1)], 0.0)
    return out


def _build_nc():
    nc = bacc.Bacc("TRN2", target_bir_lowering=False, debug=False,
                   num_devices=NCORES)
    tg_t = nc.dram_tensor("tg", [128, TGW], BF16, kind="ExternalInput")
    cst_t = nc.dram_tensor("cst", [128, 128], F32, kind="ExternalInput")
    out_t = nc.dram_tensor("out", [128, 8, NB], BF16, kind="ExternalOutput")
    tgap, cstap, oap = tg_t.ap(), cst_t.ap(), out_t.ap()

    with tile.TileContext(nc) as tc:
        with (
            tc.tile_pool(name="work", bufs=1) as work,
            tc.tile_pool(name="ps1", bufs=1, space="PSUM") as ps1,
            tc.tile_pool(name="ps2", bufs=1, space="PSUM") as ps2,
            tc.tile_pool(name="ps3", bufs=1, space="PSUM") as ps3,
            tc.tile_pool(name="ps4", bufs=1, space="PSUM") as ps4,
        ):
            # activation-table preload trigger (sqrt_and_others: Sqrt/Square/
            # Identity) while input DMA streams
            e5s = work.tile([1, 1], F32, tag="e5s")
            nc.vector.memset(e5s, BN_EPS)
            scr1 = work.tile([1, 1], F32, tag="scr1")
            nc.scalar.activation(scr1, e5s, SQRT)
            ones = work.tile([128, 128], F32, tag="ones")
            nc.vector.memset(ones, 1.0)

            tg = work.tile([128, TGW], BF16, tag="tg")
            nc.sync.dma_start(out=tg[:, 0:CHUNK], in_=tgap[:, 0:CHUNK])
            nc.sync.dma_start(out=tg[:, CHUNK:2 * CHUNK],
                              in_=tgap[:, CHUNK:2 * CHUNK])
            nc.sync.dma_start(out=tg[:, 2 * CHUNK:TGW],
                              in_=tgap[:, 2 * CHUNK:TGW])
            cst = work.tile([128, 128], F32, tag="cst")
            nc.sync.dma_start(out=cst, in_=cstap)

            def t1s(c, p):  # std toeplitz (k-folded)
                base = CHUNK * (c // 2) + 256 * (c % 2) + 128 * p
                return tg[:, base:base + 128]

            def t0s(c, p):  # mean toeplitz
                base = 2 * CHUNK + 256 * c + 128 * p
                return tg[:, base:base + 128]

            def gs(c):      # G(c): [128, B, Q+1]
                base = CHUNK * (c // 2) + 512 + 80 * (c % 2)
                return tg[:, base:base + 80].rearrange("p (b j) -> p b j", b=B)

            gsqt = work.tile([128, CPC, B, Q + 1], BF16, tag="gsqt")
            ttsq = work.tile([128, CPC, NB], F32, tag="ttsq")
            vt = work.tile([128, CPC, NB], F32, tag="vt")
            fstd = work.tile([128, CPC, NB], F32, tag="fstd")
            pack = work.tile([128, 16], F32, tag="pack")
            outt = work.tile([128, 8, NB], BF16, tag="outt")

            # per-channel-pair PSUM tiles so pair-01 consumers don't wait on
            # pair-23 conv writers (whole-tile dependency granularity)
            acc1p = [ps1.tile([128, 2, NB], F32, name=f"acc1{h}",
                               tag=f"acc1{h}") for h in range(2)]
            acc2p = [ps2.tile([128, 2, NB], F32, name=f"acc2{h}",
                               tag=f"acc2{h}") for h in range(2)]
            acc3p = [ps3.tile([128, 2, NB], F32, name=f"acc3{h}",
                               tag=f"acc3{h}") for h in range(2)]

            # gsq per chunk (bf16, 4x DVE mode)
            for h in range(2):
                cs = slice(2 * h, 2 * h + 2)
                gv = tg[:, CHUNK * h + 512:CHUNK * h + 672].rearrange(
                    "p (c b j) -> p c b j", c=2, b=B)
                nc.vector.tensor_mul(gsqt[:, cs, :, :], gv, gv)

            # std convs (acc2 = m2, acc3 = E[w2 x^2]/s2)
            for c in range(CPC):
                g = gs(c)
                gq = gsqt[:, c, :, :]
                a2 = acc2p[c // 2][:, c % 2, :]
                a3 = acc3p[c // 2][:, c % 2, :]
                nc.tensor.matmul(a2, t1s(c, 0), g[:, :, 0:Q],
                                 start=True, stop=False)
                nc.tensor.matmul(a2, t1s(c, 1), g[:, :, 1:Q + 1],
                                 start=False, stop=True)
                nc.tensor.matmul(a3, t1s(c, 0), gq[:, :, 0:Q],
                                 start=True, stop=False)
                nc.tensor.matmul(a3, t1s(c, 1), gq[:, :, 1:Q + 1],
                                 start=False, stop=True)
            # mean convs
            for c in range(CPC):
                g = gs(c)
                a1 = acc1p[c // 2][:, c % 2, :]
                nc.tensor.matmul(a1, t0s(c, 0), g[:, :, 0:Q],
                                 start=True, stop=False)
                nc.tensor.matmul(a1, t0s(c, 1), g[:, :, 1:Q + 1],
                                 start=False, stop=True)

            # ttsq = m2^2 on ACT (per channel pair), v = acc3 - m2^2 on DVE;
            # sqrt (ACT, vt->fstd) runs concurrently with the sum(v) reduce
            for h in range(2):
                cs = slice(2 * h, 2 * h + 2)
                nc.scalar.activation(ttsq[:, cs, :], acc2p[h], SQUARE)
            for h in range(2):
                cs = slice(2 * h, 2 * h + 2)
                nc.vector.tensor_sub(vt[:, cs, :], acc3p[h], ttsq[:, cs, :])
            for h in range(2):
                cs = slice(2 * h, 2 * h + 2)
                nc.scalar.activation(fstd[:, cs, :], vt[:, cs, :], SQRT)
            nc.vector.reduce_sum(out=pack[:, 12:16], in_=vt,
                                 axis=mybir.AxisListType.X)

            # mean stats (overlap with std tail): S1 via DVE reduce straight
            # from PSUM; S2 via ACT Square (h^2 -> SBUF) + DVE reduce
            fsq = work.tile([128, CPC, NB], F32, tag="fsq")
            for h in range(2):
                cs = slice(2 * h, 2 * h + 2)
                nc.scalar.activation(fsq[:, cs, :], acc1p[h], SQUARE)
            for h in range(2):
                nc.vector.reduce_sum(out=pack[:, 2 * h:2 * h + 2],
                                     in_=acc1p[h], axis=mybir.AxisListType.X)
            for h in range(2):
                cs = slice(2 * h, 2 * h + 2)
                nc.vector.reduce_sum(out=pack[:, 8 + 2 * h:10 + 2 * h],
                                     in_=fsq[:, cs, :],
                                     axis=mybir.AxisListType.X)
            for h in range(2):
                cs = slice(2 * h, 2 * h + 2)
                nc.vector.reduce_sum(out=pack[:, 4 + 2 * h:6 + 2 * h],
                                     in_=fstd[:, cs, :],
                                     axis=mybir.AxisListType.X)
            # PSUM->SBUF copy of h for the mean applies (off the pack gate)
            hsb = work.tile([128, CPC, NB], F32, tag="hsb")
            for h in range(2):
                nc.vector.tensor_copy(hsb[:, 2 * h:2 * h + 2, :], acc1p[h])

            # cross-partition reduce, replicated to all partitions, via
            # all-ones stationary matmul
            sums = ps4.tile([128, 16], F32, tag="sums")
            nc.tensor.matmul(sums, ones, pack, start=True, stop=True)

            # per-seg BN affine: X = sums/N + C  (X[:,0:8]=mu, X[:,8:16]=m2c)
            # var = m2c - mu^2 ; a = gamma/sqrt(var) = sqrt(g*|g|/var) ;
            # b = beta - mu*a.  musq = (S1/N)^2 on ACT, parallel with X.
            X = work.tile([128, 16], F32, tag="X")
            musq = work.tile([128, 8], F32, tag="musq")
            nc.scalar.activation(musq, sums[:, 0:8], SQUARE, scale=1.0 / NBT)
            nc.vector.scalar_tensor_tensor(
                out=X, in0=sums, scalar=1.0 / NBT, in1=cst[:, 0:16],
                op0=MUL, op1=ADD)
            var8 = work.tile([128, 8], F32, tag="var8")
            nc.vector.scalar_tensor_tensor(
                out=var8, in0=musq, scalar=-1.0, in1=X[:, 8:16],
                op0=MUL, op1=ADD)
            rvar = work.tile([128, 8], F32, tag="rvar")
            nc.vector.reciprocal(rvar, var8)
            q8 = work.tile([128, 8], F32, tag="q8")
            nc.vector.tensor_mul(q8, rvar, cst[:, 16:24])   # g*|g| / var
            ab = work.tile([128, 16], F32, tag="ab")
            nc.scalar.activation(ab[:, 0:8], q8, SQRT)      # a
            tmp8 = work.tile([128, 8], F32, tag="tmp8")
            nc.vector.tensor_mul(tmp8, X[:, 0:8], ab[:, 0:8])
            nc.vector.tensor_sub(ab[:, 8:16], cst[:, 24:32], tmp8)

            # applies: segs 0:4 mean (from PSUM) + seg 4 std on DVE,
            # segs 5:7 std on ACT
            for s in range(4):
                nc.vector.tensor_scalar(
                    out=outt[:, s, :], in0=hsb[:, s, :],
                    scalar1=ab[:, s:s + 1], scalar2=ab[:, 8 + s:9 + s],
                    op0=MUL, op1=ADD)
            nc.vector.tensor_scalar(
                out=outt[:, 4, :], in0=fstd[:, 0, :],
                scalar1=ab[:, 4:5], scalar2=ab[:, 12:13],
                op0=MUL, op1=ADD)
            for j in range(1, 4):
                nc.scalar.activation(outt[:, 4 + j, :], fstd[:, j, :], IDENT,
                                     bias=ab[:, 12 + j:13 + j],
                                     scale=ab[:, 4 + j:5 + j])

            nc.sync.dma_start(out=oap, in_=outt)

    nc.compile()
    return nc


_CACHE = {}


def _get_nc():
    if "nc" not in _CACHE:
        _CACHE["nc"] = _build_nc()
    return _CACHE["nc"]


def _host_prep(inputs):
    fs = np.ascontiguousarray(np.asarray(inputs["full_series"], np.float32))
    idx = np.asarray(inputs["indices"])
    starts = idx[:, 0].astype(np.int64)
    rows = (starts - W)[:, None] + np.arange(W + T)[None, :]
    bw = fs[rows]                                   # (B, 640, C)
    # G[c, kp, b, j] = bw[b, 128j + kp, c]
    G = bw.reshape(B, Q + 1, 128, C).transpose(3, 2, 0, 1)

    w1 = _soft_window_weights(np.asarray(inputs["raw_win_mean"], np.float64))
    w2 = _soft_window_weights(np.asarray(inputs["raw_win_std"], np.float64))
    s1 = w1.sum(axis=0)
    s2 = w2.sum(axis=0)
    w2k = w2 / s2                                   # fold 1/s2 into toeplitz

    gm = np.asarray(inputs["gamma_mean"], np.float64)
    bm = np.asarray(inputs["beta_mean"], np.float64)
    gs_ = np.asarray(inputs["gamma_std"], np.float64)
    bs = np.asarray(inputs["beta_std"], np.float64)

    in_maps = []
    for k in range(NCORES):
        ch = list(range(CPC * k, CPC * (k + 1)))
        tgb = np.zeros((128, TGW), np.float64)
        for i, cg in enumerate(ch):
            t1 = _toeplitz_pair(w2k[:, cg])         # (2,128,128) [p, kp, r]
            t0 = _toeplitz_pair(w1[:, cg])
            h, m = i // 2, i % 2
            base = CHUNK * h + 256 * m
            tgb[:, base:base + 256] = t1.transpose(1, 0, 2).reshape(128, 256)
            gb = CHUNK * h + 512 + 80 * m
            tgb[:, gb:gb + 80] = G[cg].reshape(128, 80)
            b0 = 2 * CHUNK + 256 * i
            tgb[:, b0:b0 + 256] = t0.transpose(1, 0, 2).reshape(128, 256)

        cstv = np.zeros(128, np.float64)
        cstv[8:12] = s1[ch] ** 2 * BN_EPS           # C for mean segs
        cstv[12:16] = BN_EPS + STD_EPS              # C for std segs
        cstv[16:20] = gm[ch] * np.abs(gm[ch])   # g*|g|: a = sqrt(g^2/var)
        cstv[20:24] = gs_[ch] * np.abs(gs_[ch])
        cstv[24:28] = bm[ch]
        cstv[28:32] = bs[ch]
        cstv[32] = STD_EPS
        cpart = np.broadcast_to(cstv[None, :], (128, 128))
        in_maps.append(dict(
            tg=np.ascontiguousarray(tgb.astype(BNP)),
            cst=np.ascontiguousarray(cpart, dtype=np.float32),
        ))
    return in_maps


def _assemble(inputs, results):
    x = np.asarray(inputs["x"], np.float32)
    full = np.empty((B, T, 3 * C), np.float32)
    full[:, :, 0:C] = x
    for k in range(NCORES):
        o = np.asarray(results[k]["out"], dtype=np.float32)
        o = o.reshape(128, 2, CPC, B, Q)
        # [r, feat, c, b, q] -> [b, q, r, c, feat] -> [b, t, c, feat]
        arr = o.transpose(3, 4, 0, 2, 1).reshape(B, T, CPC, 2)
        full[:, :, C + CPC * k:C + CPC * (k + 1)] = arr[:, :, :, 0]
        full[:, :, 2 * C + CPC * k:2 * C + CPC * (k + 1)] = arr[:, :, :, 1]
    return full


def run(inputs, trace=False):
    in_maps = _host_prep(inputs)
    nc = _get_nc()
    res = run_bass_kernel_spmd(nc, in_maps, list(range(NCORES)), trace=trace)
    return _assemble(inputs, res.results), res


def kernel(**inputs):
    out, _ = run(inputs)
    return out


# revision 27
# speedup vs baseline: 1.0184x; 1.0184x over previous
# Trainium2 Bass kernel for nn_DifferentiableFeatureLayer.
#
# Math (per reference):
#   bw[b]   = full_series[starts[b]-W : starts[b]+T]            (B, W+T, C)
#   f_mean  = conv(bw, w1)/s1 ; m2 = conv(bw, w2)/s2
#   var2    = conv(bw^2, w2)/s2 - m2^2 ; f_std = sqrt(var2 + 1e-8)
#   out     = concat([x, BN(f_mean), BN(f_std)], -1)            (B, T, 3C)
# where conv is a per-channel sliding window of length W over time and BN
# normalizes per channel over (B, T).
#
# Sharding: by channel - core k owns channels [4k, 4k+4); BN is per channel so
# cores are independent (no collectives). Host extracts the runtime-indexed
# windows and passes x through.
#
# Device compute: sliding window = banded (Toeplitz) matmul in bf16 (PSUM
# accumulates fp32):
#   acc[b, 128q+r] = sum_p sum_kp T_p[kp, r] * G[kp, b, q+p]
# The std-feature Toeplitz has 1/s2 folded in, so acc2 = m2 directly and
# acc3 = E[w2 x^2]/s2; v = acc3 - m2^2; f_std = sqrt(v + 1e-8).
# The mean feature stays in "h-units" (h = s1*f_mean): BN(h/s1) is the affine
# a*h + b with a = gamma/sqrt(var_h + s1^2*eps), b = beta - mu_h*a, so 1/s1
# only ever enters through the constant C = s1^2*eps.
#
# BN stats: per-partition partial sums (DVE reduces + fused tensor_tensor_
# reduce accumulators) -> gpsimd partition_all_reduce -> replicated [128,16]
# sums -> short per-seg affine chain -> per-seg scalars applied straight out
# of PSUM/SBUF into a bf16 output tile (DVE/ACT/Pool split).
#
# Input DMA is 3 bf16 chunks (std toeplitz+G first, then mean toeplitz,
# consts last) so std convs start while mean data is still in flight.

import numpy as np
import ml_dtypes

import concourse.bass as bass
import concourse.bacc as bacc
import concourse.tile as tile
from concourse import mybir
from concourse import bass_isa
from concourse.bass_utils import run_bass_kernel_spmd

B, T, C = 16, 512, 32
W = 128
SERIES_LEN = 100000
WIN_MIN, WIN_MAX = 2.0, 64.0
SHARP = 1.0
BN_EPS = 1e-5
STD_EPS = 1e-8

NCORES = 8
CPC = C // NCORES          # channels per core = 4
Q = T // 128               # 4 time blocks
NB = B * Q                 # 64 matmul columns
NBT = B * T                # BN population per channel
F32 = mybir.dt.float32
BF16 = mybir.dt.bfloat16
MUL = mybir.AluOpType.mult
ADD = mybir.AluOpType.add
SUB = mybir.AluOpType.subtract
SQRT = mybir.ActivationFunctionType.Sqrt
SQUARE = mybir.ActivationFunctionType.Square
IDENT = mybir.ActivationFunctionType.Identity

BNP = ml_dtypes.bfloat16

# tg blob layout (bf16, [128, 2368]):
#   chunk A1 (cols 0:672):    T1k(c0) 256 | T1k(c1) 256 | G(c0) 80 | G(c1) 80
#   chunk A2 (cols 672:1344): same for c2, c3
#   chunk B  (cols 1344:2368): T0(c0..c3), 256 each
CHUNK = 672
TGW = 2 * CHUNK + 4 * 256  # 2368


def _sigmoid(x):
    out = np.empty_like(x)
    pos = x >= 0
    out[pos] = 1.0 / (1.0 + np.exp(-x[pos]))
    ex = np.exp(x[~pos])
    out[~pos] = ex / (1.0 + ex)
    return out


def _soft_window_weights(raw):
    # (C,) -> (W, C), float64 for host-side accuracy
    win = WIN_MIN + _sigmoid(raw.astype(np.float64)) * (WIN_MAX - WIN_MIN)
    age = np.arange(W, dtype=np.float64)[::-1]
    return _sigmoid(SHARP * (win[None, :] - age[:, None]))


def _toeplitz_pair(wt):
    # wt: (W,) -> (2, 128, 128) band matrices T_p[kp, r] = wt[128p + kp - r]
    kp = np.arange(128)[:, None]
    r = np.arange(128)[None, :]
    out = np.zeros((2, 128, 128), np.float64)
    for p in range(2):
        idx = 128 * p + kp - r
        valid = (idx >= 0) & (idx < W)
        out[p] = np.where(valid, wt[np.clip(idx, 0, W - 1)], 0.0)
    return out


def _build_nc():
    nc = bacc.Bacc("TRN2", target_bir_lowering=False, debug=False,
                   num_devices=NCORES)
    tg_t = nc.dram_tensor("tg", [128, TGW], BF16, kind="ExternalInput")
    cst_t = nc.dram_tensor("cst", [128, 128], F32, kind="ExternalInput")
    out_t = nc.dram_tensor("out", [128, 8, NB], BF16, kind="ExternalOutput")
    tgap, cstap, oap = tg_t.ap(), cst_t.ap(), out_t.ap()

    with tile.TileContext(nc) as tc:
        with (
            tc.tile_pool(name="work", bufs=1) as work,
            tc.tile_pool(name="ps1", bufs=1, space="PSUM") as ps1,
            tc.tile_pool(name="ps2", bufs=1, space="PSUM") as ps2,
            tc.tile_pool(name="ps3", bufs=1, space="PSUM") as ps3,
            tc.tile_pool(name="ps4", bufs=1, space="PSUM") as ps4,
        ):
            # activation-table preload trigger (sqrt_and_others: Sqrt/Square/
            # Identity) while input DMA streams
            e5s = work.tile([1, 1], F32, tag="e5s")
            nc.vector.memset(e5s, BN_EPS)
            scr1 = work.tile([1, 1], F32, tag="scr1")
            nc.scalar.activation(scr1, e5s, SQRT)
            ones = work.tile([128, 128], F32, tag="ones")
            nc.vector.memset(ones, 1.0)

            tg = work.tile([128, TGW], BF16, tag="tg")
            nc.sync.dma_start(out=tg[:, 0:CHUNK], in_=tgap[:, 0:CHUNK])
            nc.sync.dma_start(out=tg[:, CHUNK:2 * CHUNK],
                              in_=tgap[:, CHUNK:2 * CHUNK])
            nc.sync.dma_start(out=tg[:, 2 * CHUNK:TGW],
                              in_=tgap[:, 2 * CHUNK:TGW])
            cst = work.tile([128, 128], F32, tag="cst")
            nc.sync.dma_start(out=cst, in_=cstap)

            def t1s(c, p):  # std toeplitz (k-folded)
                base = CHUNK * (c // 2) + 256 * (c % 2) + 128 * p
                return tg[:, base:base + 128]

            def t0s(c, p):  # mean toeplitz
                base = 2 * CHUNK + 256 * c + 128 * p
                return tg[:, base:base + 128]

            def gs(c):      # G(c): [128, B, Q+1]
                base = CHUNK * (c // 2) + 512 + 80 * (c % 2)
                return tg[:, base:base + 80].rearrange("p (b j) -> p b j", b=B)

            gsqt = work.tile([128, CPC, B, Q + 1], BF16, tag="gsqt")
            ttsq = work.tile([128, CPC, NB], F32, tag="ttsq")
            vt = work.tile([128, CPC, NB], F32, tag="vt")
            fstd = work.tile([128, CPC, NB], F32, tag="fstd")
            pack = work.tile([128, 16], F32, tag="pack")
            outt = work.tile([128, 8, NB], BF16, tag="outt")

            # per-channel-pair PSUM tiles so pair-01 consumers don't wait on
            # pair-23 conv writers (whole-tile dependency granularity)
            acc1p = [ps1.tile([128, 2, NB], F32, name=f"acc1{h}",
                               tag=f"acc1{h}") for h in range(2)]
            acc2p = [ps2.tile([128, 2, NB], F32, name=f"acc2{h}",
                               tag=f"acc2{h}") for h in range(2)]
            acc3p = [ps3.tile([128, 2, NB], F32, name=f"acc3{h}",
                               tag=f"acc3{h}") for h in range(2)]

            # gsq per chunk (bf16, 4x DVE mode)
            for h in range(2):
                cs = slice(2 * h, 2 * h + 2)
                gv = tg[:, CHUNK * h + 512:CHUNK * h + 672].rearrange(
                    "p (c b j) -> p c b j", c=2, b=B)
                nc.vector.tensor_mul(gsqt[:, cs, :, :], gv, gv)

            # std convs (acc2 = m2, acc3 = E[w2 x^2]/s2)
            for c in range(CPC):
                g = gs(c)
                gq = gsqt[:, c, :, :]
                a2 = acc2p[c // 2][:, c % 2, :]
                a3 = acc3p[c // 2][:, c % 2, :]
                nc.tensor.matmul(a2, t1s(c, 0), g[:, :, 0:Q],
                                 start=True, stop=False)
                nc.tensor.matmul(a2, t1s(c, 1), g[:, :, 1:Q + 1],
                                 start=False, stop=True)
                nc.tensor.matmul(a3, t1s(c, 0), gq[:, :, 0:Q],
                                 start=True, stop=False)
                nc.tensor.matmul(a3, t1s(c, 1), gq[:, :, 1:Q + 1],
                                 start=False, stop=True)
            # mean convs
            for c in range(CPC):
                g = gs(c)
                a1 = acc1p[c // 2][:, c % 2, :]
                nc.tensor.matmul(a1, t0s(c, 0), g[:, :, 0:Q],
                                 start=True, stop=False)
                nc.tensor.matmul(a1, t0s(c, 1), g[:, :, 1:Q + 1],
                                 start=False, stop=True)

            # ttsq = m2^2 on ACT (per channel pair), v = acc3 - m2^2 on DVE;
            # sqrt (ACT, vt->fstd) runs concurrently with the sum(v) reduce
            for h in range(2):
                cs = slice(2 * h, 2 * h + 2)
                nc.scalar.activation(ttsq[:, cs, :], acc2p[h], SQUARE)
            for h in range(2):
                cs = slice(2 * h, 2 * h + 2)
                nc.vector.tensor_sub(vt[:, cs, :], acc3p[h], ttsq[:, cs, :])
            for h in range(2):
                cs = slice(2 * h, 2 * h + 2)
                nc.scalar.activation(fstd[:, cs, :], vt[:, cs, :], SQRT)
            for h in range(2):
                cs = slice(2 * h, 2 * h + 2)
                nc.vector.reduce_sum(out=pack[:, 12 + 2 * h:14 + 2 * h],
                                     in_=vt[:, cs, :],
                                     axis=mybir.AxisListType.X)

            # mean stats (overlap with std tail): S1 via DVE reduce straight
            # from PSUM; S2 via ACT Square (h^2 -> SBUF) + DVE reduce
            fsq = work.tile([128, CPC, NB], F32, tag="fsq")
            for h in range(2):
                cs = slice(2 * h, 2 * h + 2)
                nc.scalar.activation(fsq[:, cs, :], acc1p[h], SQUARE)
            for h in range(2):
                nc.vector.reduce_sum(out=pack[:, 2 * h:2 * h + 2],
                                     in_=acc1p[h], axis=mybir.AxisListType.X)
            for h in range(2):
                cs = slice(2 * h, 2 * h + 2)
                nc.vector.reduce_sum(out=pack[:, 4 + 2 * h:6 + 2 * h],
                                     in_=fstd[:, cs, :],
                                     axis=mybir.AxisListType.X)
            for h in range(2):
                cs = slice(2 * h, 2 * h + 2)
                nc.vector.reduce_sum(out=pack[:, 8 + 2 * h:10 + 2 * h],
                                     in_=fsq[:, cs, :],
                                     axis=mybir.AxisListType.X)
            # PSUM->SBUF copy of h for the mean applies, on ACT (idle there;
            # keeps the DVE queue free for the pack reduces)
            hsb = work.tile([128, CPC, NB], F32, tag="hsb")
            for h in range(2):
                nc.scalar.activation(hsb[:, 2 * h:2 * h + 2, :], acc1p[h],
                                     mybir.ActivationFunctionType.Copy)

            # cross-partition reduce, replicated to all partitions, via
            # all-ones stationary matmul
            sums = ps4.tile([128, 16], F32, tag="sums")
            nc.tensor.matmul(sums, ones, pack, start=True, stop=True)

            # per-seg BN affine: X = sums/N + C  (X[:,0:8]=mu, X[:,8:16]=m2c)
            # var = m2c - mu^2 ; a = gamma/sqrt(var) = sqrt(g*|g|/var) ;
            # b = beta - mu*a.  musq = (S1/N)^2 on ACT, parallel with X.
            X = work.tile([128, 16], F32, tag="X")
            musq = work.tile([128, 8], F32, tag="musq")
            nc.scalar.activation(musq, sums[:, 0:8], SQUARE, scale=1.0 / NBT)
            nc.vector.scalar_tensor_tensor(
                out=X, in0=sums, scalar=1.0 / NBT, in1=cst[:, 0:16],
                op0=MUL, op1=ADD)
            var8 = work.tile([128, 8], F32, tag="var8")
            nc.vector.scalar_tensor_tensor(
                out=var8, in0=musq, scalar=-1.0, in1=X[:, 8:16],
                op0=MUL, op1=ADD)
            rvar = work.tile([128, 8], F32, tag="rvar")
            nc.vector.reciprocal(rvar, var8)
            q8 = work.tile([128, 8], F32, tag="q8")
            nc.vector.tensor_mul(q8, rvar, cst[:, 16:24])   # g*|g| / var
            ab = work.tile([128, 16], F32, tag="ab")
            nc.scalar.activation(ab[:, 0:8], q8, SQRT)      # a
            tmp8 = work.tile([128, 8], F32, tag="tmp8")
            nc.vector.tensor_mul(tmp8, X[:, 0:8], ab[:, 0:8])
            nc.vector.tensor_sub(ab[:, 8:16], cst[:, 24:32], tmp8)

            # applies: segs 0:4 mean (from PSUM) + seg 4 std on DVE,
            # segs 5:7 std on ACT
            for s in range(4):
                nc.vector.tensor_scalar(
                    out=outt[:, s, :], in0=hsb[:, s, :],
                    scalar1=ab[:, s:s + 1], scalar2=ab[:, 8 + s:9 + s],
                    op0=MUL, op1=ADD)
            for j in range(2):
                nc.vector.tensor_scalar(
                    out=outt[:, 4 + j, :], in0=fstd[:, j, :],
                    scalar1=ab[:, 4 + j:5 + j], scalar2=ab[:, 12 + j:13 + j],
                    op0=MUL, op1=ADD)
            for j in range(2, 4):
                nc.scalar.activation(outt[:, 4 + j, :], fstd[:, j, :], IDENT,
                                     bias=ab[:, 12 + j:13 + j],
                                     scale=ab[:, 4 + j:5 + j])

            nc.sync.dma_start(out=oap, in_=outt)

    nc.compile()
    return nc


_CACHE = {}


def _get_nc():
    if "nc" not in _CACHE:
        _CACHE["nc"] = _build_nc()
    return _CACHE["nc"]


def _host_prep(inputs):
    fs = np.ascontiguousarray(np.asarray(inputs["full_series"], np.float32))
    idx = np.asarray(inputs["indices"])
    starts = idx[:, 0].astype(np.int64)
    rows = (starts - W)[:, None] + np.arange(W + T)[None, :]
    bw = fs[rows]                                   # (B, 640, C)
    # G[c, kp, b, j] = bw[b, 128j + kp, c]
    G = bw.reshape(B, Q + 1, 128, C).transpose(3, 2, 0, 1)

    w1 = _soft_window_weights(np.asarray(inputs["raw_win_mean"], np.float64))
    w2 = _soft_window_weights(np.asarray(inputs["raw_win_std"], np.float64))
    s1 = w1.sum(axis=0)
    s2 = w2.sum(axis=0)
    w2k = w2 / s2                                   # fold 1/s2 into toeplitz

    gm = np.asarray(inputs["gamma_mean"], np.float64)
    bm = np.asarray(inputs["beta_mean"], np.float64)
    gs_ = np.asarray(inputs["gamma_std"], np.float64)
    bs = np.asarray(inputs["beta_std"], np.float64)

    in_maps = []
    for k in range(NCORES):
        ch = list(range(CPC * k, CPC * (k + 1)))
        tgb = np.zeros((128, TGW), np.float64)
        for i, cg in enumerate(ch):
            t1 = _toeplitz_pair(w2k[:, cg])         # (2,128,128) [p, kp, r]
            t0 = _toeplitz_pair(w1[:, cg])
            h, m = i // 2, i % 2
            base = CHUNK * h + 256 * m
            tgb[:, base:base + 256] = t1.transpose(1, 0, 2).reshape(128, 256)
            gb = CHUNK * h + 512 + 80 * m
            tgb[:, gb:gb + 80] = G[cg].reshape(128, 80)
            b0 = 2 * CHUNK + 256 * i
            tgb[:, b0:b0 + 256] = t0.transpose(1, 0, 2).reshape(128, 256)

        cstv = np.zeros(128, np.float64)
        cstv[8:12] = s1[ch] ** 2 * BN_EPS           # C for mean segs
        cstv[12:16] = BN_EPS + STD_EPS              # C for std segs
        cstv[16:20] = gm[ch] * np.abs(gm[ch])   # g*|g|: a = sqrt(g^2/var)
        cstv[20:24] = gs_[ch] * np.abs(gs_[ch])
        cstv[24:28] = bm[ch]
        cstv[28:32] = bs[ch]
        cstv[32] = STD_EPS
        cpart = np.broadcast_to(cstv[None, :], (128, 128))
        in_maps.append(dict(
            tg=np.ascontiguousarray(tgb.astype(BNP)),
            cst=np.ascontiguousarray(cpart, dtype=np.float32),
        ))
    return in_maps


def _assemble(inputs, results):
    x = np.asarray(inputs["x"], np.float32)
    full = np.empty((B, T, 3 * C), np.float32)
    full[:, :, 0:C] = x
    for k in range(NCORES):
        o = np.asarray(results[k]["out"], dtype=np.float32)
        o = o.reshape(128, 2, CPC, B, Q)
        # [r, feat, c, b, q] -> [b, q, r, c, feat] -> [b, t, c, feat]
        arr = o.transpose(3, 4, 0, 2, 1).reshape(B, T, CPC, 2)
        full[:, :, C + CPC * k:C + CPC * (k + 1)] = arr[:, :, :, 0]
        full[:, :, 2 * C + CPC * k:2 * C + CPC * (k + 1)] = arr[:, :, :, 1]
    return full


def run(inputs, trace=False):
    in_maps = _host_prep(inputs)
    nc = _get_nc()
    res = run_bass_kernel_spmd(nc, in_maps, list(range(NCORES)), trace=trace)
    return _assemble(inputs, res.results), res


def kernel(**inputs):
    out, _ = run(inputs)
    return out


# revision 30
# speedup vs baseline: 1.0197x; 1.0013x over previous
# Trainium2 Bass kernel for nn_DifferentiableFeatureLayer.
#
# Math (per reference):
#   bw[b]   = full_series[starts[b]-W : starts[b]+T]            (B, W+T, C)
#   f_mean  = conv(bw, w1)/s1 ; m2 = conv(bw, w2)/s2
#   var2    = conv(bw^2, w2)/s2 - m2^2 ; f_std = sqrt(var2 + 1e-8)
#   out     = concat([x, BN(f_mean), BN(f_std)], -1)            (B, T, 3C)
# where conv is a per-channel sliding window of length W over time and BN
# normalizes per channel over (B, T).
#
# Sharding: by channel - core k owns channels [4k, 4k+4); BN is per channel so
# cores are independent (no collectives). Host extracts the runtime-indexed
# windows and passes x through.
#
# Device compute: sliding window = banded (Toeplitz) matmul in bf16 (PSUM
# accumulates fp32):
#   acc[b, 128q+r] = sum_p sum_kp T_p[kp, r] * G[kp, b, q+p]
# The std-feature Toeplitz has 1/s2 folded in, so acc2 = m2 directly and
# acc3 = E[w2 x^2]/s2; v = acc3 - m2^2; f_std = sqrt(v + 1e-8).
# The mean feature stays in "h-units" (h = s1*f_mean): BN(h/s1) is the affine
# a*h + b with a = gamma/sqrt(var_h + s1^2*eps), b = beta - mu_h*a, so 1/s1
# only ever enters through the constant C = s1^2*eps.
#
# BN stats: per-partition partial sums (DVE reduces + fused tensor_tensor_
# reduce accumulators) -> gpsimd partition_all_reduce -> replicated [128,16]
# sums -> short per-seg affine chain -> per-seg scalars applied straight out
# of PSUM/SBUF into a bf16 output tile (DVE/ACT/Pool split).
#
# Input DMA is 3 bf16 chunks (std toeplitz+G first, then mean toeplitz,
# consts last) so std convs start while mean data is still in flight.

import numpy as np
import ml_dtypes

import concourse.bass as bass
import concourse.bacc as bacc
import concourse.tile as tile
from concourse import mybir
from concourse import bass_isa
from concourse.bass_utils import run_bass_kernel_spmd

B, T, C = 16, 512, 32
W = 128
SERIES_LEN = 100000
WIN_MIN, WIN_MAX = 2.0, 64.0
SHARP = 1.0
BN_EPS = 1e-5
STD_EPS = 1e-8

NCORES = 8
CPC = C // NCORES          # channels per core = 4
Q = T // 128               # 4 time blocks
NB = B * Q                 # 64 matmul columns
NBT = B * T                # BN population per channel
F32 = mybir.dt.float32
BF16 = mybir.dt.bfloat16
MUL = mybir.AluOpType.mult
ADD = mybir.AluOpType.add
SUB = mybir.AluOpType.subtract
SQRT = mybir.ActivationFunctionType.Sqrt
SQUARE = mybir.ActivationFunctionType.Square
IDENT = mybir.ActivationFunctionType.Identity

BNP = ml_dtypes.bfloat16

# tg blob layout (bf16, [128, 2368]):
#   chunk A1 (cols 0:672):    T1k(c0) 256 | T1k(c1) 256 | G(c0) 80 | G(c1) 80
#   chunk A2 (cols 672:1344): same for c2, c3
#   chunk B  (cols 1344:2368): T0(c0..c3), 256 each
CHUNK = 672
TGW = 2 * CHUNK + 4 * 256  # 2368


def _sigmoid(x):
    out = np.empty_like(x)
    pos = x >= 0
    out[pos] = 1.0 / (1.0 + np.exp(-x[pos]))
    ex = np.exp(x[~pos])
    out[~pos] = ex / (1.0 + ex)
    return out


def _soft_window_weights(raw):
    # (C,) -> (W, C), float64 for host-side accuracy
    win = WIN_MIN + _sigmoid(raw.astype(np.float64)) * (WIN_MAX - WIN_MIN)
    age = np.arange(W, dtype=np.float64)[::-1]
    return _sigmoid(SHARP * (win[None, :] - age[:, None]))


def _toeplitz_pair(wt):
    # wt: (W,) -> (2, 128, 128) band matrices T_p[kp, r] = wt[128p + kp - r]
    kp = np.arange(128)[:, None]
    r = np.arange(128)[None, :]
    out = np.zeros((2, 128, 128), np.float64)
    for p in range(2):
        idx = 128 * p + kp - r
        valid = (idx >= 0) & (idx < W)
        out[p] = np.where(valid, wt[np.clip(idx, 0, W - 1)], 0.0)
    return out


def _build_nc():
    nc = bacc.Bacc("TRN2", target_bir_lowering=False, debug=False,
                   num_devices=NCORES)
    tg_t = nc.dram_tensor("tg", [128, TGW], BF16, kind="ExternalInput")
    cst_t = nc.dram_tensor("cst", [128, 128], F32, kind="ExternalInput")
    out_t = nc.dram_tensor("out", [128, 8, NB], BF16, kind="ExternalOutput")
    tgap, cstap, oap = tg_t.ap(), cst_t.ap(), out_t.ap()

    with tile.TileContext(nc) as tc:
        with (
            tc.tile_pool(name="work", bufs=1) as work,
            tc.tile_pool(name="ps1", bufs=1, space="PSUM") as ps1,
            tc.tile_pool(name="ps2", bufs=1, space="PSUM") as ps2,
            tc.tile_pool(name="ps3", bufs=1, space="PSUM") as ps3,
            tc.tile_pool(name="ps4", bufs=1, space="PSUM") as ps4,
        ):
            # activation-table preload trigger (sqrt_and_others: Sqrt/Square/
            # Identity) while input DMA streams
            e5s = work.tile([1, 1], F32, tag="e5s")
            nc.vector.memset(e5s, BN_EPS)
            scr1 = work.tile([1, 1], F32, tag="scr1")
            nc.scalar.activation(scr1, e5s, SQRT)
            # reduction stationaries: ones/N folds the 1/NBT scaling into the
            # cross-partition matmul; onesC/128 adds the per-seg C constants
            ones = work.tile([128, 128], F32, tag="ones")
            nc.vector.memset(ones, 1.0 / NBT)
            onesc = work.tile([128, 128], F32, tag="onesc")
            nc.vector.memset(onesc, 1.0 / 128.0)

            tg = work.tile([128, TGW], BF16, tag="tg")
            nc.sync.dma_start(out=tg[:, 0:CHUNK], in_=tgap[:, 0:CHUNK])
            nc.sync.dma_start(out=tg[:, CHUNK:2 * CHUNK],
                              in_=tgap[:, CHUNK:2 * CHUNK])
            nc.sync.dma_start(out=tg[:, 2 * CHUNK:TGW],
                              in_=tgap[:, 2 * CHUNK:TGW])
            cst = work.tile([128, 128], F32, tag="cst")
            nc.sync.dma_start(out=cst, in_=cstap)

            def t1s(c, p):  # std toeplitz (k-folded)
                base = CHUNK * (c // 2) + 256 * (c % 2) + 128 * p
                return tg[:, base:base + 128]

            def t0s(c, p):  # mean toeplitz
                base = 2 * CHUNK + 256 * c + 128 * p
                return tg[:, base:base + 128]

            def gs(c):      # G(c): [128, B, Q+1]
                base = CHUNK * (c // 2) + 512 + 80 * (c % 2)
                return tg[:, base:base + 80].rearrange("p (b j) -> p b j", b=B)

            gsqt = work.tile([128, CPC, B, Q + 1], BF16, tag="gsqt")
            ttsq = work.tile([128, CPC, NB], F32, tag="ttsq")
            vt = work.tile([128, CPC, NB], F32, tag="vt")
            fstd = work.tile([128, CPC, NB], F32, tag="fstd")
            pack = work.tile([128, 16], F32, tag="pack")
            outt = work.tile([128, 8, NB], BF16, tag="outt")

            # per-channel-pair PSUM tiles so pair-01 consumers don't wait on
            # pair-23 conv writers (whole-tile dependency granularity)
            acc1p = [ps1.tile([128, 2, NB], F32, name=f"acc1{h}",
                               tag=f"acc1{h}") for h in range(2)]
            acc2p = [ps2.tile([128, 2, NB], F32, name=f"acc2{h}",
                               tag=f"acc2{h}") for h in range(2)]
            acc3p = [ps3.tile([128, 2, NB], F32, name=f"acc3{h}",
                               tag=f"acc3{h}") for h in range(2)]

            # gsq per chunk (bf16, 4x DVE mode)
            for h in range(2):
                cs = slice(2 * h, 2 * h + 2)
                gv = tg[:, CHUNK * h + 512:CHUNK * h + 672].rearrange(
                    "p (c b j) -> p c b j", c=2, b=B)
                nc.vector.tensor_mul(gsqt[:, cs, :, :], gv, gv)

            # std convs (acc2 = m2, acc3 = E[w2 x^2]/s2)
            for c in range(CPC):
                g = gs(c)
                gq = gsqt[:, c, :, :]
                a2 = acc2p[c // 2][:, c % 2, :]
                a3 = acc3p[c // 2][:, c % 2, :]
                nc.tensor.matmul(a2, t1s(c, 0), g[:, :, 0:Q],
                                 start=True, stop=False)
                nc.tensor.matmul(a2, t1s(c, 1), g[:, :, 1:Q + 1],
                                 start=False, stop=True)
                nc.tensor.matmul(a3, t1s(c, 0), gq[:, :, 0:Q],
                                 start=True, stop=False)
                nc.tensor.matmul(a3, t1s(c, 1), gq[:, :, 1:Q + 1],
                                 start=False, stop=True)
            # mean convs
            for c in range(CPC):
                g = gs(c)
                a1 = acc1p[c // 2][:, c % 2, :]
                nc.tensor.matmul(a1, t0s(c, 0), g[:, :, 0:Q],
                                 start=True, stop=False)
                nc.tensor.matmul(a1, t0s(c, 1), g[:, :, 1:Q + 1],
                                 start=False, stop=True)

            # ttsq = m2^2 on ACT (per channel pair), v = acc3 - m2^2 on DVE;
            # sqrt (ACT, vt->fstd) runs concurrently with the sum(v) reduce
            for h in range(2):
                cs = slice(2 * h, 2 * h + 2)
                nc.scalar.activation(ttsq[:, cs, :], acc2p[h], SQUARE)
            for h in range(2):
                cs = slice(2 * h, 2 * h + 2)
                nc.vector.tensor_sub(vt[:, cs, :], acc3p[h], ttsq[:, cs, :])
            for h in range(2):
                cs = slice(2 * h, 2 * h + 2)
                nc.scalar.activation(fstd[:, cs, :], vt[:, cs, :], SQRT)
            # stats reduces in data-readiness order; mean S2 = h^2 via ACT
            # Square (acc1 PSUM -> fsq SBUF) + DVE reduce
            fsq = work.tile([128, CPC, NB], F32, tag="fsq")
            for h in range(2):
                cs = slice(2 * h, 2 * h + 2)
                nc.scalar.activation(fsq[:, cs, :], acc1p[h], SQUARE)
            for h in range(2):
                cs = slice(2 * h, 2 * h + 2)
                nc.vector.reduce_sum(out=pack[:, 12 + 2 * h:14 + 2 * h],
                                     in_=vt[:, cs, :],
                                     axis=mybir.AxisListType.X)
                nc.vector.reduce_sum(out=pack[:, 4 + 2 * h:6 + 2 * h],
                                     in_=fstd[:, cs, :],
                                     axis=mybir.AxisListType.X)
                nc.vector.reduce_sum(out=pack[:, 2 * h:2 * h + 2],
                                     in_=acc1p[h], axis=mybir.AxisListType.X)
                nc.vector.reduce_sum(out=pack[:, 8 + 2 * h:10 + 2 * h],
                                     in_=fsq[:, cs, :],
                                     axis=mybir.AxisListType.X)
            # PSUM->SBUF copy of h for the mean applies, on ACT (idle there;
            # keeps the DVE queue free for the pack reduces)
            hsb = work.tile([128, CPC, NB], F32, tag="hsb")
            for h in range(2):
                nc.scalar.activation(hsb[:, 2 * h:2 * h + 2, :], acc1p[h],
                                     mybir.ActivationFunctionType.Copy)

            # cross-partition reduce via all-ones/N stationary matmul (sums
            # land pre-scaled: mu | m2), plus C constants via onesC x Crow:
            # sums[:,0:8] = mu, sums[:,8:16] = S2/N + C = m2c, replicated
            sums = ps4.tile([128, 16], F32, tag="sums")
            nc.tensor.matmul(sums, ones, pack, start=True, stop=False)
            nc.tensor.matmul(sums, onesc, cst[:, 0:16], start=False,
                             stop=True)

            # per-seg BN affine: var = m2c - mu^2 ;
            # a = gamma/sqrt(var) = sqrt(g*|g|/var) ; b = beta - mu*a
            musq = work.tile([128, 8], F32, tag="musq")
            nc.scalar.activation(musq, sums[:, 0:8], SQUARE)
            var8 = work.tile([128, 8], F32, tag="var8")
            nc.vector.scalar_tensor_tensor(
                out=var8, in0=musq, scalar=-1.0, in1=sums[:, 8:16],
                op0=MUL, op1=ADD)
            rvar = work.tile([128, 8], F32, tag="rvar")
            nc.vector.reciprocal(rvar, var8)
            q8 = work.tile([128, 8], F32, tag="q8")
            nc.vector.tensor_mul(q8, rvar, cst[:, 16:24])   # g*|g| / var
            ab = work.tile([128, 16], F32, tag="ab")
            nc.scalar.activation(ab[:, 0:8], q8, SQRT)      # a
            tmp8 = work.tile([128, 8], F32, tag="tmp8")
            nc.vector.scalar_tensor_tensor(
                out=tmp8, in0=sums[:, 0:8], scalar=-1.0, in1=ab[:, 0:8],
                op0=MUL, op1=MUL)                            # -mu*a
            nc.vector.tensor_add(ab[:, 8:16], cst[:, 24:32], tmp8)

            # applies: segs 0:4 mean (from PSUM) + seg 4 std on DVE,
            # segs 5:7 std on ACT
            for s in range(4):
                nc.vector.tensor_scalar(
                    out=outt[:, s, :], in0=hsb[:, s, :],
                    scalar1=ab[:, s:s + 1], scalar2=ab[:, 8 + s:9 + s],
                    op0=MUL, op1=ADD)
            for j in range(2):
                nc.vector.tensor_scalar(
                    out=outt[:, 4 + j, :], in0=fstd[:, j, :],
                    scalar1=ab[:, 4 + j:5 + j], scalar2=ab[:, 12 + j:13 + j],
                    op0=MUL, op1=ADD)
            for j in range(2, 4):
                nc.scalar.activation(outt[:, 4 + j, :], fstd[:, j, :], IDENT,
                                     bias=ab[:, 12 + j:13 + j],
                                     scale=ab[:, 4 + j:5 + j])

            nc.sync.dma_start(out=oap, in_=outt)

    nc.compile()
    return nc


_CACHE = {}


def _get_nc():
    if "nc" not in _CACHE:
        _CACHE["nc"] = _build_nc()
    return _CACHE["nc"]


def _host_prep(inputs):
    fs = np.ascontiguousarray(np.asarray(inputs["full_series"], np.float32))
    idx = np.asarray(inputs["indices"])
    starts = idx[:, 0].astype(np.int64)
    rows = (starts - W)[:, None] + np.arange(W + T)[None, :]
    bw = fs[rows]                                   # (B, 640, C)
    # G[c, kp, b, j] = bw[b, 128j + kp, c]
    G = bw.reshape(B, Q + 1, 128, C).transpose(3, 2, 0, 1)

    w1 = _soft_window_weights(np.asarray(inputs["raw_win_mean"], np.float64))
    w2 = _soft_window_weights(np.asarray(inputs["raw_win_std"], np.float64))
    s1 = w1.sum(axis=0)
    s2 = w2.sum(axis=0)
    w2k = w2 / s2                                   # fold 1/s2 into toeplitz

    gm = np.asarray(inputs["gamma_mean"], np.float64)
    bm = np.asarray(inputs["beta_mean"], np.float64)
    gs_ = np.asarray(inputs["gamma_std"], np.float64)
    bs = np.asarray(inputs["beta_std"], np.float64)

    in_maps = []
    for k in range(NCORES):
        ch = list(range(CPC * k, CPC * (k + 1)))
        tgb = np.zeros((128, TGW), np.float64)
        for i, cg in enumerate(ch):
            t1 = _toeplitz_pair(w2k[:, cg])         # (2,128,128) [p, kp, r]
            t0 = _toeplitz_pair(w1[:, cg])
            h, m = i // 2, i % 2
            base = CHUNK * h + 256 * m
            tgb[:, base:base + 256] = t1.transpose(1, 0, 2).reshape(128, 256)
            gb = CHUNK * h + 512 + 80 * m
            tgb[:, gb:gb + 80] = G[cg].reshape(128, 80)
            b0 = 2 * CHUNK + 256 * i
            tgb[:, b0:b0 + 256] = t0.transpose(1, 0, 2).reshape(128, 256)

        cstv = np.zeros(128, np.float64)
        cstv[8:12] = s1[ch] ** 2 * BN_EPS           # C for mean segs
        cstv[12:16] = BN_EPS + STD_EPS              # C for std segs
        cstv[16:20] = gm[ch] * np.abs(gm[ch])   # g*|g|: a = sqrt(g^2/var)
        cstv[20:24] = gs_[ch] * np.abs(gs_[ch])
        cstv[24:28] = bm[ch]
        cstv[28:32] = bs[ch]
        cstv[32] = STD_EPS
        cpart = np.broadcast_to(cstv[None, :], (128, 128))
        in_maps.append(dict(
            tg=np.ascontiguousarray(tgb.astype(BNP)),
            cst=np.ascontiguousarray(cpart, dtype=np.float32),
        ))
    return in_maps


def _assemble(inputs, results):
    x = np.asarray(inputs["x"], np.float32)
    full = np.empty((B, T, 3 * C), np.float32)
    full[:, :, 0:C] = x
    for k in range(NCORES):
        o = np.asarray(results[k]["out"], dtype=np.float32)
        o = o.reshape(128, 2, CPC, B, Q)
        # [r, feat, c, b, q] -> [b, q, r, c, feat] -> [b, t, c, feat]
        arr = o.transpose(3, 4, 0, 2, 1).reshape(B, T, CPC, 2)
        full[:, :, C + CPC * k:C + CPC * (k + 1)] = arr[:, :, :, 0]
        full[:, :, 2 * C + CPC * k:2 * C + CPC * (k + 1)] = arr[:, :, :, 1]
    return full


def run(inputs, trace=False):
    in_maps = _host_prep(inputs)
    nc = _get_nc()
    res = run_bass_kernel_spmd(nc, in_maps, list(range(NCORES)), trace=trace)
    return _assemble(inputs, res.results), res


def kernel(**inputs):
    out, _ = run(inputs)
    return out


# revision 38
# speedup vs baseline: 1.0409x; 1.0208x over previous
# Trainium2 Bass kernel for nn_DifferentiableFeatureLayer.
#
# Math (per reference):
#   bw[b]   = full_series[starts[b]-W : starts[b]+T]            (B, W+T, C)
#   f_mean  = conv(bw, w1)/s1 ; m2 = conv(bw, w2)/s2
#   var2    = conv(bw^2, w2)/s2 - m2^2 ; f_std = sqrt(var2 + 1e-8)
#   out     = concat([x, BN(f_mean), BN(f_std)], -1)            (B, T, 3C)
# where conv is a per-channel sliding window of length W over time and BN
# normalizes per channel over (B, T).
#
# Sharding: by channel - core k owns channels [4k, 4k+4); BN is per channel so
# cores are independent (no collectives). Host extracts the runtime-indexed
# windows and passes x through.
#
# Device compute: sliding window = banded (Toeplitz) matmul in bf16 (PSUM
# accumulates fp32):
#   acc[b, 128q+r] = sum_p sum_kp T_p[kp, r] * G[kp, b, q+p]
# The std-feature Toeplitz has 1/s2 folded in, so acc2 = m2 directly and
# acc3 = E[w2 x^2]/s2; v = acc3 - m2^2; f_std = sqrt(v + 1e-8).
# The mean feature stays in "h-units" (h = s1*f_mean): BN(h/s1) is the affine
# a*h + b with a = gamma/sqrt(var_h + s1^2*eps), b = beta - mu_h*a, so 1/s1
# only ever enters through the constant C = s1^2*eps.
#
# BN stats: per-partition partial sums (DVE reduces + fused tensor_tensor_
# reduce accumulators) -> gpsimd partition_all_reduce -> replicated [128,16]
# sums -> short per-seg affine chain -> per-seg scalars applied straight out
# of PSUM/SBUF into a bf16 output tile (DVE/ACT/Pool split).
#
# Input DMA is 3 bf16 chunks (std toeplitz+G first, then mean toeplitz,
# consts last) so std convs start while mean data is still in flight.

import numpy as np
import ml_dtypes

import concourse.bass as bass
import concourse.bacc as bacc
import concourse.tile as tile
from concourse import mybir
from concourse import bass_isa
from concourse.bass_utils import run_bass_kernel_spmd

B, T, C = 16, 512, 32
W = 128
SERIES_LEN = 100000
WIN_MIN, WIN_MAX = 2.0, 64.0
SHARP = 1.0
BN_EPS = 1e-5
STD_EPS = 1e-8

NCORES = 8
CPC = C // NCORES          # channels per core = 4
Q = T // 128               # 4 time blocks
NB = B * Q                 # 64 matmul columns
NBT = B * T                # BN population per channel
F32 = mybir.dt.float32
BF16 = mybir.dt.bfloat16
MUL = mybir.AluOpType.mult
ADD = mybir.AluOpType.add
SUB = mybir.AluOpType.subtract
SQRT = mybir.ActivationFunctionType.Sqrt
SQUARE = mybir.ActivationFunctionType.Square
IDENT = mybir.ActivationFunctionType.Identity

BNP = ml_dtypes.bfloat16

# tg blob layout (bf16, [128, 2368]):
#   chunk A1 (cols 0:832):     T1k(c0) 256 | T1k(c1) 256 | G(c0..c3) 4x80
#   chunk A2 (cols 832:1344):  T1k(c2) 256 | T1k(c3) 256
#   chunk B  (cols 1344:2368): T0(c0..c3), 256 each
A1W = 832
A2W = 512
BBASE = A1W + A2W          # 1344
TGW = BBASE + 4 * 256      # 2368


def _t1col(c):
    return 256 * c if c < 2 else A1W + 256 * (c - 2)


def _gcol(c):
    return 512 + 80 * c


def _t0col(c):
    return BBASE + 256 * c


def _sigmoid(x):
    out = np.empty_like(x)
    pos = x >= 0
    out[pos] = 1.0 / (1.0 + np.exp(-x[pos]))
    ex = np.exp(x[~pos])
    out[~pos] = ex / (1.0 + ex)
    return out


def _soft_window_weights(raw):
    # (C,) -> (W, C), float64 for host-side accuracy
    win = WIN_MIN + _sigmoid(raw.astype(np.float64)) * (WIN_MAX - WIN_MIN)
    age = np.arange(W, dtype=np.float64)[::-1]
    return _sigmoid(SHARP * (win[None, :] - age[:, None]))


def _toeplitz_pair(wt):
    # wt: (W,) -> (2, 128, 128) band matrices T_p[kp, r] = wt[128p + kp - r]
    kp = np.arange(128)[:, None]
    r = np.arange(128)[None, :]
    out = np.zeros((2, 128, 128), np.float64)
    for p in range(2):
        idx = 128 * p + kp - r
        valid = (idx >= 0) & (idx < W)
        out[p] = np.where(valid, wt[np.clip(idx, 0, W - 1)], 0.0)
    return out


def _build_nc(bfast=True):
    nc = bacc.Bacc("TRN2", target_bir_lowering=False, debug=False,
                   num_devices=NCORES)
    tg_t = nc.dram_tensor("tg", [128, TGW], BF16, kind="ExternalInput")
    cst_t = nc.dram_tensor("cst", [128, 128], F32, kind="ExternalInput")
    out_t = nc.dram_tensor("out", [128, 8, NB], BF16, kind="ExternalOutput")
    tgap, cstap, oap = tg_t.ap(), cst_t.ap(), out_t.ap()

    with tile.TileContext(nc) as tc:
        with (
            tc.tile_pool(name="work", bufs=1) as work,
            tc.tile_pool(name="ps1", bufs=1, space="PSUM") as ps1,
            tc.tile_pool(name="ps2", bufs=1, space="PSUM") as ps2,
            tc.tile_pool(name="ps3", bufs=1, space="PSUM") as ps3,
            tc.tile_pool(name="ps4", bufs=1, space="PSUM") as ps4,
        ):
            # activation-table preload trigger (sqrt_and_others: Sqrt/Square/
            # Identity) while input DMA streams
            e5s = work.tile([1, 1], F32, tag="e5s")
            nc.vector.memset(e5s, BN_EPS)
            scr1 = work.tile([1, 1], F32, tag="scr1")
            nc.scalar.activation(scr1, e5s, SQRT)
            # reduction stationaries: ones/N folds the 1/NBT scaling into the
            # cross-partition matmul; onesC/128 adds the per-seg C constants
            ones = work.tile([128, 128], F32, tag="ones")
            nc.vector.memset(ones, 1.0 / NBT)
            onesc = work.tile([128, 128], F32, tag="onesc")
            nc.vector.memset(onesc, 1.0 / 128.0)

            tg = work.tile([128, TGW], BF16, tag="tg")
            nc.sync.dma_start(out=tg[:, 0:A1W], in_=tgap[:, 0:A1W])
            nc.sync.dma_start(out=tg[:, A1W:BBASE], in_=tgap[:, A1W:BBASE])
            nc.sync.dma_start(out=tg[:, BBASE:TGW], in_=tgap[:, BBASE:TGW])
            cst = work.tile([128, 128], F32, tag="cst")
            nc.sync.dma_start(out=cst, in_=cstap)

            def t1s(c, p):  # std toeplitz (k-folded)
                base = _t1col(c) + 128 * p
                return tg[:, base:base + 128]

            def t0s(c, p):  # mean toeplitz
                base = _t0col(c) + 128 * p
                return tg[:, base:base + 128]

            def gs(c):      # G(c): [128, B, Q+1]
                base = _gcol(c)
                return tg[:, base:base + 80].rearrange("p (b j) -> p b j", b=B)

            gsqt = work.tile([128, CPC, B, Q + 1], BF16, tag="gsqt")
            ttsq = work.tile([128, CPC, NB], F32, tag="ttsq")
            vt = work.tile([128, CPC, NB], F32, tag="vt")
            fstd = work.tile([128, CPC, NB], F32, tag="fstd")
            pack = work.tile([128, 16], F32, tag="pack")
            outt = work.tile([128, 8, NB], BF16, tag="outt")

            # per-channel-pair PSUM tiles so pair-01 consumers don't wait on
            # pair-23 conv writers (whole-tile dependency granularity)
            acc1p = [ps1.tile([128, 2, NB], F32, name=f"acc1{h}",
                               tag=f"acc1{h}") for h in range(2)]
            acc2p = [ps2.tile([128, 2, NB], F32, name=f"acc2{h}",
                               tag=f"acc2{h}") for h in range(2)]
            acc3p = [ps3.tile([128, 2, NB], F32, name=f"acc3{h}",
                               tag=f"acc3{h}") for h in range(2)]

            # gsq for all channels (bf16, 4x DVE mode)
            gv = tg[:, 512:832].rearrange("p (c b j) -> p c b j", c=CPC, b=B)
            nc.vector.tensor_mul(gsqt, gv, gv)

            # std convs (acc2 = m2, acc3 = E[w2 x^2]/s2); acc2 of a pair
            # fully before acc3 so the ACT square isn't queued behind acc3
            for h in range(2):
                for c in (2 * h, 2 * h + 1):
                    g = gs(c)
                    a2 = acc2p[h][:, c % 2, :]
                    nc.tensor.matmul(a2, t1s(c, 0), g[:, :, 0:Q],
                                     start=True, stop=False)
                    nc.tensor.matmul(a2, t1s(c, 1), g[:, :, 1:Q + 1],
                                     start=False, stop=True)
                for c in (2 * h, 2 * h + 1):
                    gq = gsqt[:, c, :, :]
                    a3 = acc3p[h][:, c % 2, :]
                    nc.tensor.matmul(a3, t1s(c, 0), gq[:, :, 0:Q],
                                     start=True, stop=False)
                    nc.tensor.matmul(a3, t1s(c, 1), gq[:, :, 1:Q + 1],
                                     start=False, stop=True)
            # mean convs
            for c in range(CPC):
                g = gs(c)
                a1 = acc1p[c // 2][:, c % 2, :]
                nc.tensor.matmul(a1, t0s(c, 0), g[:, :, 0:Q],
                                 start=True, stop=False)
                nc.tensor.matmul(a1, t0s(c, 1), g[:, :, 1:Q + 1],
                                 start=False, stop=True)

            # ttsq = m2^2 on ACT (per channel pair), v = acc3 - m2^2 on DVE;
            # sqrt (ACT, vt->fstd) runs concurrently with the sum(v) reduce
            for h in range(2):
                cs = slice(2 * h, 2 * h + 2)
                nc.scalar.activation(ttsq[:, cs, :], acc2p[h], SQUARE)
            for h in range(2):
                cs = slice(2 * h, 2 * h + 2)
                nc.vector.tensor_sub(vt[:, cs, :], acc3p[h], ttsq[:, cs, :])
            for h in range(2):
                cs = slice(2 * h, 2 * h + 2)
                nc.scalar.activation(fstd[:, cs, :], vt[:, cs, :], SQRT)
            # stats reduces in data-readiness order; mean S2 = h^2 via ACT
            # Square (acc1 PSUM -> fsq SBUF) + DVE reduce
            fsq = work.tile([128, CPC, NB], F32, tag="fsq")
            for h in range(2):
                cs = slice(2 * h, 2 * h + 2)
                nc.scalar.activation(fsq[:, cs, :], acc1p[h], SQUARE)
            for h in range(2):
                cs = slice(2 * h, 2 * h + 2)
                nc.vector.reduce_sum(out=pack[:, 12 + 2 * h:14 + 2 * h],
                                     in_=vt[:, cs, :],
                                     axis=mybir.AxisListType.X)
                nc.vector.reduce_sum(out=pack[:, 4 + 2 * h:6 + 2 * h],
                                     in_=fstd[:, cs, :],
                                     axis=mybir.AxisListType.X)
                nc.vector.reduce_sum(out=pack[:, 2 * h:2 * h + 2],
                                     in_=acc1p[h], axis=mybir.AxisListType.X)
                nc.vector.reduce_sum(out=pack[:, 8 + 2 * h:10 + 2 * h],
                                     in_=fsq[:, cs, :],
                                     axis=mybir.AxisListType.X)
            # PSUM->SBUF copy of h for the mean applies, on ACT (idle there;
            # keeps the DVE queue free for the pack reduces)
            hsb = work.tile([128, CPC, NB], F32, tag="hsb")
            for h in range(2):
                nc.scalar.activation(hsb[:, 2 * h:2 * h + 2, :], acc1p[h],
                                     mybir.ActivationFunctionType.Copy)

            # cross-partition reduce via all-ones/N stationary matmul (sums
            # land pre-scaled: mu | m2), plus C constants via onesC x Crow:
            # sums[:,0:8] = mu, sums[:,8:16] = S2/N + C = m2c, replicated
            sums = ps4.tile([128, 16], F32, tag="sums")
            nc.tensor.matmul(sums, ones, pack, start=True, stop=False)
            nc.tensor.matmul(sums, onesc, cst[:, 0:16], start=False,
                             stop=True)

            # per-seg BN affine, all-DVE except one ACT sqrt:
            # var = m2c - mu^2 ; a = gamma/sqrt(var) = sqrt(g*|g|/var) ;
            # b = beta - mu*a (when beta==0: b = -mu*a in one fused op)
            ssb = work.tile([128, 16], F32, tag="ssb")
            nc.vector.tensor_copy(ssb, sums)
            musq = work.tile([128, 8], F32, tag="musq")
            nc.vector.tensor_mul(musq, ssb[:, 0:8], ssb[:, 0:8])
            var8 = work.tile([128, 8], F32, tag="var8")
            nc.vector.scalar_tensor_tensor(
                out=var8, in0=musq, scalar=-1.0, in1=ssb[:, 8:16],
                op0=MUL, op1=ADD)
            rvar = work.tile([128, 8], F32, tag="rvar")
            nc.vector.reciprocal(rvar, var8)
            q8 = work.tile([128, 8], F32, tag="q8")
            nc.vector.tensor_mul(q8, rvar, cst[:, 16:24])   # g*|g| / var
            ab = work.tile([128, 16], F32, tag="ab")
            nc.scalar.activation(ab[:, 0:8], q8, SQRT)      # a
            if bfast:
                nc.vector.scalar_tensor_tensor(
                    out=ab[:, 8:16], in0=ssb[:, 0:8], scalar=-1.0,
                    in1=ab[:, 0:8], op0=MUL, op1=MUL)        # b = -mu*a
            else:
                tmp8 = work.tile([128, 8], F32, tag="tmp8")
                nc.vector.scalar_tensor_tensor(
                    out=tmp8, in0=ssb[:, 0:8], scalar=-1.0, in1=ab[:, 0:8],
                    op0=MUL, op1=MUL)                        # -mu*a
                nc.vector.tensor_add(ab[:, 8:16], cst[:, 24:32], tmp8)

            # applies: segs 0:4 mean (from PSUM) + seg 4 std on DVE,
            # segs 5:7 std on ACT
            for s in range(4):
                nc.vector.tensor_scalar(
                    out=outt[:, s, :], in0=hsb[:, s, :],
                    scalar1=ab[:, s:s + 1], scalar2=ab[:, 8 + s:9 + s],
                    op0=MUL, op1=ADD)
            for j in range(2):
                nc.vector.tensor_scalar(
                    out=outt[:, 4 + j, :], in0=fstd[:, j, :],
                    scalar1=ab[:, 4 + j:5 + j], scalar2=ab[:, 12 + j:13 + j],
                    op0=MUL, op1=ADD)
            for j in range(2, 4):
                nc.scalar.activation(outt[:, 4 + j, :], fstd[:, j, :], IDENT,
                                     bias=ab[:, 12 + j:13 + j],
                                     scale=ab[:, 4 + j:5 + j])

            nc.sync.dma_start(out=oap, in_=outt)

    nc.compile()
    return nc


_CACHE = {}


def _get_nc(bfast=True):
    key = ("nc", bfast)
    if key not in _CACHE:
        _CACHE[key] = _build_nc(bfast)
    return _CACHE[key]


def _host_prep(inputs):
    fs = np.ascontiguousarray(np.asarray(inputs["full_series"], np.float32))
    idx = np.asarray(inputs["indices"])
    starts = idx[:, 0].astype(np.int64)
    rows = (starts - W)[:, None] + np.arange(W + T)[None, :]
    bw = fs[rows]                                   # (B, 640, C)
    # G[c, kp, b, j] = bw[b, 128j + kp, c]
    G = bw.reshape(B, Q + 1, 128, C).transpose(3, 2, 0, 1)

    w1 = _soft_window_weights(np.asarray(inputs["raw_win_mean"], np.float64))
    w2 = _soft_window_weights(np.asarray(inputs["raw_win_std"], np.float64))
    s1 = w1.sum(axis=0)
    s2 = w2.sum(axis=0)
    w2k = w2 / s2                                   # fold 1/s2 into toeplitz

    gm = np.asarray(inputs["gamma_mean"], np.float64)
    bm = np.asarray(inputs["beta_mean"], np.float64)
    gs_ = np.asarray(inputs["gamma_std"], np.float64)
    bs = np.asarray(inputs["beta_std"], np.float64)

    in_maps = []
    for k in range(NCORES):
        ch = list(range(CPC * k, CPC * (k + 1)))
        tgb = np.zeros((128, TGW), np.float64)
        for i, cg in enumerate(ch):
            t1 = _toeplitz_pair(w2k[:, cg])         # (2,128,128) [p, kp, r]
            t0 = _toeplitz_pair(w1[:, cg])
            base = _t1col(i)
            tgb[:, base:base + 256] = t1.transpose(1, 0, 2).reshape(128, 256)
            gb = _gcol(i)
            tgb[:, gb:gb + 80] = G[cg].reshape(128, 80)
            b0 = _t0col(i)
            tgb[:, b0:b0 + 256] = t0.transpose(1, 0, 2).reshape(128, 256)

        cstv = np.zeros(128, np.float64)
        cstv[8:12] = s1[ch] ** 2 * BN_EPS           # C for mean segs
        cstv[12:16] = BN_EPS + STD_EPS              # C for std segs
        cstv[16:20] = gm[ch] * np.abs(gm[ch])   # g*|g|: a = sqrt(g^2/var)
        cstv[20:24] = gs_[ch] * np.abs(gs_[ch])
        cstv[24:28] = bm[ch]
        cstv[28:32] = bs[ch]
        cstv[32] = STD_EPS
        cpart = np.broadcast_to(cstv[None, :], (128, 128))
        in_maps.append(dict(
            tg=np.ascontiguousarray(tgb.astype(BNP)),
            cst=np.ascontiguousarray(cpart, dtype=np.float32),
        ))
    return in_maps


def _assemble(inputs, results):
    x = np.asarray(inputs["x"], np.float32)
    full = np.empty((B, T, 3 * C), np.float32)
    full[:, :, 0:C] = x
    for k in range(NCORES):
        o = np.asarray(results[k]["out"], dtype=np.float32)
        o = o.reshape(128, 2, CPC, B, Q)
        # [r, feat, c, b, q] -> [b, q, r, c, feat] -> [b, t, c, feat]
        arr = o.transpose(3, 4, 0, 2, 1).reshape(B, T, CPC, 2)
        full[:, :, C + CPC * k:C + CPC * (k + 1)] = arr[:, :, :, 0]
        full[:, :, 2 * C + CPC * k:2 * C + CPC * (k + 1)] = arr[:, :, :, 1]
    return full


def run(inputs, trace=False):
    in_maps = _host_prep(inputs)
    bfast = bool(np.all(np.asarray(inputs["beta_mean"]) == 0)
                 and np.all(np.asarray(inputs["beta_std"]) == 0))
    nc = _get_nc(bfast)
    res = run_bass_kernel_spmd(nc, in_maps, list(range(NCORES)), trace=trace)
    return _assemble(inputs, res.results), res


def kernel(**inputs):
    out, _ = run(inputs)
    return out
